# revision 24
# baseline (speedup 1.0000x reference)
"""Performer (FAVOR+ linear attention) Trainium2 kernel, 8-core SPMD.

Sharding: core c -> (batch b = c//2, head-group g = c%2 of 6 heads).
Each core computes its 6 heads end-to-end plus a partial output
projection; the host sums the two head-group partials per batch and
adds bproj.

Math simplifications vs the reference (exact up to ~1e-11 rel):
  - qf = exp(proj_q): the exp(-|xs_q|^2/2) and 1/sqrt(M) factors on the
    q side cancel between numerator and denominator D.
  - kf's 1/sqrt(M) also cancels; exp(-|xs_k|^2/2) is folded into v
    (and into the ones-column that produces ksum).
  - EPS=1e-8 on D is dropped (D is >> EPS here).
"""

from contextlib import ExitStack

import ml_dtypes
import numpy as np

import concourse.bacc as bacc
import concourse.tile as tile
from concourse import mybir
from concourse.masks import make_identity

B, N, DIM = 4, 2048, 768
H, C, M = 12, 64, 384
SCALE = float(C ** (-0.25))
HL = H // 2          # heads per core
PAIRS = HL // 2      # packed head-pairs per core
NCH = N // 128       # 16 n-chunks
DT = DIM // 128      # 6 dim-tiles
MT = M // 128        # 3 feature-tiles
NF = N // 512        # 4 free-dim chunks

F32 = mybir.dt.float32
R32 = mybir.dt.float32r
BF16 = mybir.dt.bfloat16

_CACHED = {}


def _r(ap):
    """fp32r view for fp32 matmul operands; bf16/fp32r pass through."""
    return ap.bitcast(R32) if ap.dtype == F32 else ap


def build_nc():
    nc = bacc.Bacc("TRN2", target_bir_lowering=False, debug=False)

    xT = nc.dram_tensor("xT", [DIM, N], R32, kind="ExternalInput")
    Wqk = nc.dram_tensor("Wqk", [DIM, 2 * HL * C], R32, kind="ExternalInput")
    Wv = nc.dram_tensor("Wv", [DIM, HL * C], R32, kind="ExternalInput")
    Wm = nc.dram_tensor("Wm", [HL, C, M], R32, kind="ExternalInput")
    Wp = nc.dram_tensor("Wp", [HL * C, DIM], R32, kind="ExternalInput")
    OUTS = [
        nc.dram_tensor(f"OUT{p}", [N, DIM], F32, kind="ExternalOutput")
        for p in range(PAIRS)
    ]

    with tile.TileContext(nc) as tc:
        _build_program(tc, nc, xT, Wqk, Wv, Wm, Wp, OUTS)

    nc.compile()
    return nc


def _build_program(tc, nc, xT, Wqk, Wv, Wm, Wp, OUTS):
    ctx = ExitStack()
    with ctx:
        # ---------- persistent pools ----------
        singles = ctx.enter_context(tc.tile_pool(name="singles", bufs=1))
        outp = ctx.enter_context(tc.tile_pool(name="outp", bufs=2))

        ident = singles.tile([C + 1, C + 1], F32, tag="ident", name="ident")
        make_identity(nc, ident)
        cst_f = singles.tile([128, 1], F32, tag="cst_f", name="cst_f")
        nc.vector.memset(cst_f[:], 1.0)
        # rows 64-127 pair with the kT^2 half of the stacked phi-k operand
        cstk = singles.tile([128, M], F32, tag="cstk", name="cstk")
        nc.vector.memset(cstk[:], -0.5 * SCALE * SCALE)

        ws2 = singles.tile([128, HL, M], R32, tag="ws2", name="ws2")
        wp_sb = singles.tile([128, PAIRS, DIM], R32, tag="wp", name="wp_sb")

        qkT = [
            singles.tile([128, N], R32, tag=f"qkT{i}", name=f"qkT{i}")
            for i in range(2 * PAIRS)
        ]  # 0..2 = q pairs, 3..5 = k pairs
        vz = [
            singles.tile([128, HL, C + 1], R32, tag=f"vz{ch}", name=f"vz{ch}")
            for ch in range(NCH)
        ]

        pp_proj = ctx.enter_context(
            tc.tile_pool(name="pp_proj", bufs=3, space="PSUM")
        )
        pp_misc = ctx.enter_context(
            tc.tile_pool(name="pp_misc", bufs=2, space="PSUM")
        )

        # ---------- phase 1: qkv projection ----------
        with tc.tile_pool(name="ph1", bufs=1) as ph1:
            xt = [
                ph1.tile([128, N], R32, tag=f"xt{d}", name=f"xt{d}")
                for d in range(DT)
            ]
            wqk_sb = [
                ph1.tile([128, 2 * HL * C], R32, tag=f"wqk{d}", name=f"wqk{d}")
                for d in range(DT)
            ]
            wv_sb = [
                ph1.tile([128, HL * C], R32, tag=f"wv{d}", name=f"wv{d}")
                for d in range(DT)
            ]
            for d in range(DT):
                nc.sync.dma_start(out=xt[d][:], in_=xT[d * 128:(d + 1) * 128, :])
                nc.sync.dma_start(out=wqk_sb[d][:], in_=Wqk[d * 128:(d + 1) * 128, :])
            for d in range(DT):
                nc.sync.dma_start(out=wv_sb[d][:], in_=Wv[d * 128:(d + 1) * 128, :])
            for h in range(HL):
                nc.sync.dma_start(out=ws2[0:C, h, :], in_=Wm[h])
                nc.sync.dma_start(out=ws2[C:128, h, :], in_=Wm[h])
            for p in range(PAIRS):
                nc.sync.dma_start(
                    out=wp_sb[:, p, :], in_=Wp[p * 128:(p + 1) * 128, :]
                )

            # q/k head-pairs: qkT[i] = Wqk[:, 128i:128i+128].T @ x.T  (128, N)
            for i in range(2 * PAIRS):
                ps_qk = [
                    pp_proj.tile([128, 1024], F32, tag="proj", name=f"ps_qk{j}")
                    for j in range(2)
                ]
                for d in range(DT):
                    for nf in range(NF):
                        nc.tensor.matmul(
                            ps_qk[nf // 2][:, (nf % 2) * 512:(nf % 2 + 1) * 512],
                            _r(wqk_sb[d][:, i * 128:(i + 1) * 128]),
                            _r(xt[d][:, nf * 512:(nf + 1) * 512]),
                            start=(d == 0),
                            stop=(d == DT - 1),
                        )
                for j in range(2):
                    nc.vector.tensor_copy(
                        qkT[i][:, j * 1024:(j + 1) * 1024], ps_qk[j][:]
                    )

            # v in natural layout (+ ones column for ksum)
            for ch in range(NCH):
                ps_v = pp_proj.tile([128, HL * C], F32, tag="proj", name="ps_v")
                for d in range(DT):
                    nc.tensor.matmul(
                        ps_v[:],
                        _r(xt[d][:, ch * 128:(ch + 1) * 128]),
                        _r(wv_sb[d][:]),
                        start=(d == 0),
                        stop=(d == DT - 1),
                    )
                nc.vector.tensor_copy(
                    vz[ch][:, :, 0:C],
                    ps_v.rearrange("p (h c) -> p h c", h=HL),
                )
                nc.vector.tensor_copy(
                    vz[ch][:, :, C:C + 1],
                    cst_f[:, None, 0:1].broadcast_to([128, HL, 1]),
                )

        # ---------- phase 2: per-head Performer attention ----------
        with tc.tile_pool(name="ph2", bufs=1) as ph2, \
             tc.tile_pool(name="sp2", bufs=2) as sp2, \
             tc.tile_pool(name="atp", bufs=2) as atp, \
             tc.tile_pool(name="dp1", bufs=1) as dp1:
            attnT_cur = None
            for h in [1, 0, 3, 2, 5, 4]:
                pair, half = h // 2, h % 2
                if half == 1:
                    attnT_cur = atp.tile(
                        [128, N], R32, tag="attnT", name="attnT"
                    )
                b0 = 64 * half
                qT_h = qkT[pair][b0:b0 + 64, :]
                kT_h = qkT[PAIRS + pair][b0:b0 + 64, :]
                ws_h = ws2[b0:b0 + 64, h, :]

                # stacked phi-k operand: rows 0-63 = kT_h, rows 64-127 = kT_h^2
                # (pairs with wsk rows 64-127 = -SCALE^2/2 to fuse the -|xs|^2/2
                # bias into the projection matmul)
                sq_tmp = ph2.tile([128, N], R32, tag="sqt", name="sq_tmp")
                nc.vector.tensor_mul(sq_tmp[b0:b0 + 64, :], kT_h, kT_h)
                stack = sp2.tile([128, N], R32, tag="stack", name="stack")
                nc.sync.dma_start(out=stack[0:64, :], in_=kT_h)
                nc.sync.dma_start(
                    out=stack[64:128, :], in_=sq_tmp[b0:b0 + 64, :]
                )
                wsk = sp2.tile([128, M], R32, tag="wsk", name="wsk")
                nc.vector.tensor_copy(wsk[0:64, :], ws2[0:C, h, :])
                nc.vector.tensor_copy(wsk[64:128, :], cstk[64:128, :])

                # phi_q transposed: qfT[:, mt, :] = exp(ws_h[:, mt].T @ qT_h)
                qfT = ph2.tile([128, MT, N], R32, tag="qfT", name="qfT")
                for mt in range(MT):
                    for nfh in range(2):
                        ps_q = pp_proj.tile(
                            [128, 1024], F32, tag="proj", name="ps_q"
                        )
                        for j in range(2):
                            nf = 2 * nfh + j
                            nc.tensor.matmul(
                                ps_q[:, j * 512:(j + 1) * 512],
                                _r(ws_h[:, mt * 128:(mt + 1) * 128]),
                                _r(qT_h[:, nf * 512:(nf + 1) * 512]),
                                start=True, stop=True,
                            )
                        nc.scalar.activation(
                            qfT[:, mt, nfh * 1024:(nfh + 1) * 1024],
                            ps_q[:],
                            mybir.ActivationFunctionType.Exp,
                        )

                # phi_k natural (with fused -|xs|^2/2):
                # kf[:, ch, :] = exp(stack[:, ch].T @ wsk)
                kf = ph2.tile([128, NCH, M], R32, tag="kf", name="kf")
                for ch2 in range(NCH // 2):
                    ps_k = pp_proj.tile(
                        [128, 2, 512], F32, tag="proj", name="ps_k"
                    )
                    for j in range(2):
                        ch = 2 * ch2 + j
                        nc.tensor.matmul(
                            ps_k[:, j, 0:M],
                            _r(stack[:, ch * 128:(ch + 1) * 128]),
                            _r(wsk[:]),
                            start=True, stop=True,
                        )
                    nc.scalar.activation(
                        kf[:, 2 * ch2:2 * ch2 + 2, :],
                        ps_k[:, :, 0:M],
                        mybir.ActivationFunctionType.Exp,
                    )

                # ktvT = sum_ch vz[ch].T @ kf[ch]   (C+1, M)
                pktv = pp_misc.tile([C + 1, M], F32, tag="misc", name="pktv")
                for ch in range(NCH):
                    nc.tensor.matmul(
                        pktv[:],
                        _r(vz[ch][:, h, :]),
                        _r(kf[:, ch, :]),
                        start=(ch == 0),
                        stop=(ch == NCH - 1),
                    )
                ktvTs = ph2.tile([C + 1, M], F32, tag="ktvTs", name="ktvTs")
                nc.vector.tensor_copy(ktvTs[:], pktv[:])

                # transpose to natural ktv (M, C+1) as MT (128, C+1) tiles
                ktv = ph2.tile([128, MT, C + 1], R32, tag="ktv_n", name="ktv")
                for mt in range(MT):
                    pt = pp_misc.tile([128, C + 1], F32, tag="misc", name="pt_tr")
                    nc.tensor.transpose(
                        pt[:],
                        ktvTs[:, mt * 128:(mt + 1) * 128],
                        ident[:],
                    )
                    nc.vector.tensor_copy(ktv[:, mt, :], pt[:])

                # numerator+D, divide, pack into attnT
                attn_dst = (
                    attnT_cur[0:64, :]
                    if half == 0
                    else ph2.tile([64, N], R32, tag="attn_tmp", name="attn_tmp")
                )
                for nfh in range(2):
                    nt = pp_proj.tile([C + 1, 1024], F32, tag="proj", name="nt")
                    for mt in range(MT):
                        for j in range(2):
                            nf = 2 * nfh + j
                            nc.tensor.matmul(
                                nt[:, j * 512:(j + 1) * 512],
                                _r(ktv[:, mt, :]),
                                _r(qfT[:, mt, nf * 512:(nf + 1) * 512]),
                                start=(mt == 0),
                                stop=(mt == MT - 1),
                            )
                    rec = dp1.tile([1, 1024], F32, tag="rec", name="rec")
                    nc.vector.tensor_copy(rec[0:1, :], nt[64:65, :])
                    nc.vector.reciprocal_approx_fast(rec[0:1, :], rec[0:1, :])
                    dbs = dp1.tile([64, 1024], F32, tag="dbs", name="dbs")
                    nc.gpsimd.partition_broadcast(dbs[:], rec[0:1, :])
                    nc.vector.tensor_mul(
                        attn_dst[:, nfh * 1024:(nfh + 1) * 1024],
                        nt[0:64, :],
                        dbs[:],
                    )
                if half == 1:
                    nc.sync.dma_start(
                        out=attnT_cur[64:128, :], in_=attn_dst[:]
                    )
                else:
                    # pair complete: project this pair's heads into OUT{pair}
                    for ch in range(NCH):
                        ot = outp.tile([128, DIM], F32, tag="ot", name="ot")
                        for hf in range(2):
                            po = pp_proj.tile(
                                [128, 384], F32, tag="proj", name="po"
                            )
                            nc.tensor.matmul(
                                po[:],
                                attnT_cur[:, ch * 128:(ch + 1) * 128],
                                _r(wp_sb[:, pair, hf * 384:(hf + 1) * 384]),
                                start=True, stop=True,
                            )
                            nc.vector.tensor_copy(
                                ot[:, hf * 384:(hf + 1) * 384], po[:]
                            )
                        nc.sync.dma_start(
                            out=OUTS[pair][ch * 128:(ch + 1) * 128, :],
                            in_=ot[:],
                        )


def _host_prep(x, w, Wqkv, Wproj):
    in_maps = []
    for core in range(8):
        b, g = core // 2, core % 2
        xTb = np.ascontiguousarray(x[b].T)
        Wq = Wqkv[:, g * 384:(g + 1) * 384]
        Wk = Wqkv[:, DIM + g * 384:DIM + (g + 1) * 384]
        in_maps.append(
            {
                "xT": xTb.astype(np.float32),
                "Wqk": np.ascontiguousarray(
                    np.concatenate([Wq, Wk], axis=1), dtype=np.float32
                ),
                "Wv": np.ascontiguousarray(
                    Wqkv[:, 2 * DIM + g * 384:2 * DIM + (g + 1) * 384],
                    dtype=np.float32,
                ),
                "Wm": np.ascontiguousarray(
                    w[g * HL:(g + 1) * HL] * SCALE, dtype=np.float32
                ),
                "Wp": np.ascontiguousarray(
                    Wproj[g * 384:(g + 1) * 384, :], dtype=np.float32
                ),
            }
        )
    return in_maps


def kernel(x, w, Wqkv, Wproj, bproj, _trace=False):
    from concourse.bass_utils import run_bass_kernel_spmd

    if "nc" not in _CACHED:
        _CACHED["nc"] = build_nc()
    nc = _CACHED["nc"]

    in_maps = _host_prep(
        np.asarray(x, np.float32),
        np.asarray(w, np.float32),
        np.asarray(Wqkv, np.float32),
        np.asarray(Wproj, np.float32),
    )
    res = run_bass_kernel_spmd(nc, in_maps, list(range(8)), trace=_trace)
    _CACHED["last_result"] = res

    out = np.empty((B, N, DIM), np.float32)
    for b in range(B):
        out[b] = sum(
            res.results[2 * b + g][f"OUT{p}"]
            for g in range(2) for p in range(PAIRS)
        )
    out += np.asarray(bproj, np.float32)[None, None, :]
    return out


# revision 25
# speedup vs baseline: 1.0191x; 1.0191x over previous
"""Performer (FAVOR+ linear attention) Trainium2 kernel, 8-core SPMD.

Sharding: core c -> (batch b = c//2, head-group g = c%2 of 6 heads).
Each core computes its 6 heads end-to-end plus a partial output
projection; the host sums the two head-group partials per batch and
adds bproj.

Math simplifications vs the reference (exact up to ~1e-11 rel):
  - qf = exp(proj_q): the exp(-|xs_q|^2/2) and 1/sqrt(M) factors on the
    q side cancel between numerator and denominator D.
  - kf's 1/sqrt(M) also cancels; exp(-|xs_k|^2/2) is folded into v
    (and into the ones-column that produces ksum).
  - EPS=1e-8 on D is dropped (D is >> EPS here).
"""

from contextlib import ExitStack

import ml_dtypes
import numpy as np

import concourse.bacc as bacc
import concourse.tile as tile
from concourse import mybir
from concourse.masks import make_identity

B, N, DIM = 4, 2048, 768
H, C, M = 12, 64, 384
SCALE = float(C ** (-0.25))
HL = H // 2          # heads per core
PAIRS = HL // 2      # packed head-pairs per core
NCH = N // 128       # 16 n-chunks
DT = DIM // 128      # 6 dim-tiles
MT = M // 128        # 3 feature-tiles
NF = N // 512        # 4 free-dim chunks

F32 = mybir.dt.float32
R32 = mybir.dt.float32r
BF16 = mybir.dt.bfloat16

_CACHED = {}


def _r(ap):
    """fp32r view for fp32 matmul operands; bf16/fp32r pass through."""
    return ap.bitcast(R32) if ap.dtype == F32 else ap


def build_nc():
    nc = bacc.Bacc("TRN2", target_bir_lowering=False, debug=False)

    xT = nc.dram_tensor("xT", [DIM, N], R32, kind="ExternalInput")
    Wqk = nc.dram_tensor("Wqk", [DIM, 2 * HL * C], R32, kind="ExternalInput")
    Wv = nc.dram_tensor("Wv", [DIM, HL * C], R32, kind="ExternalInput")
    Wm = nc.dram_tensor("Wm", [HL, C, M], R32, kind="ExternalInput")
    Wp = nc.dram_tensor("Wp", [HL * C, DIM], R32, kind="ExternalInput")
    OUTS = [
        nc.dram_tensor(f"OUT{p}", [N, DIM], F32, kind="ExternalOutput")
        for p in range(PAIRS)
    ]

    with tile.TileContext(nc) as tc:
        _build_program(tc, nc, xT, Wqk, Wv, Wm, Wp, OUTS)

    nc.compile()
    return nc


def _build_program(tc, nc, xT, Wqk, Wv, Wm, Wp, OUTS):
    ctx = ExitStack()
    with ctx:
        # ---------- persistent pools ----------
        singles = ctx.enter_context(tc.tile_pool(name="singles", bufs=1))
        outp = ctx.enter_context(tc.tile_pool(name="outp", bufs=2))

        ident = singles.tile([C + 1, C + 1], F32, tag="ident", name="ident")
        make_identity(nc, ident)
        cst_f = singles.tile([128, 1], F32, tag="cst_f", name="cst_f")
        nc.vector.memset(cst_f[:], 1.0)
        # rows 64-127 pair with the kT^2 half of the stacked phi-k operand
        cstk = singles.tile([128, M], F32, tag="cstk", name="cstk")
        nc.vector.memset(cstk[:], -0.5 * SCALE * SCALE)

        ws2 = singles.tile([128, HL, M], R32, tag="ws2", name="ws2")
        wp_sb = singles.tile([128, PAIRS, DIM], R32, tag="wp", name="wp_sb")

        qkT = [
            singles.tile([128, N], R32, tag=f"qkT{i}", name=f"qkT{i}")
            for i in range(2 * PAIRS)
        ]  # 0..2 = q pairs, 3..5 = k pairs
        vz = [
            singles.tile([128, HL, C + 1], R32, tag=f"vz{ch}", name=f"vz{ch}")
            for ch in range(NCH)
        ]

        pp_proj = ctx.enter_context(
            tc.tile_pool(name="pp_proj", bufs=3, space="PSUM")
        )
        pp_misc = ctx.enter_context(
            tc.tile_pool(name="pp_misc", bufs=2, space="PSUM")
        )

        # ---------- phase 1: qkv projection ----------
        with tc.tile_pool(name="ph1", bufs=1) as ph1:
            xt = [
                ph1.tile([128, N], R32, tag=f"xt{d}", name=f"xt{d}")
                for d in range(DT)
            ]
            wqk_sb = [
                ph1.tile([128, 2 * HL * C], R32, tag=f"wqk{d}", name=f"wqk{d}")
                for d in range(DT)
            ]
            wv_sb = [
                ph1.tile([128, HL * C], R32, tag=f"wv{d}", name=f"wv{d}")
                for d in range(DT)
            ]
            for d in range(DT):
                nc.sync.dma_start(out=xt[d][:], in_=xT[d * 128:(d + 1) * 128, :])
                nc.sync.dma_start(out=wqk_sb[d][:], in_=Wqk[d * 128:(d + 1) * 128, :])
            for d in range(DT):
                nc.sync.dma_start(out=wv_sb[d][:], in_=Wv[d * 128:(d + 1) * 128, :])
            for h in range(HL):
                nc.sync.dma_start(out=ws2[0:C, h, :], in_=Wm[h])
                nc.sync.dma_start(out=ws2[C:128, h, :], in_=Wm[h])
            for p in range(PAIRS):
                nc.sync.dma_start(
                    out=wp_sb[:, p, :], in_=Wp[p * 128:(p + 1) * 128, :]
                )

            # q/k head-pairs: qkT[i] = Wqk[:, 128i:128i+128].T @ x.T  (128, N)
            for i in range(2 * PAIRS):
                ps_qk = [
                    pp_proj.tile([128, 1024], F32, tag="proj", name=f"ps_qk{j}")
                    for j in range(2)
                ]
                for d in range(DT):
                    for nf in range(NF):
                        nc.tensor.matmul(
                            ps_qk[nf // 2][:, (nf % 2) * 512:(nf % 2 + 1) * 512],
                            _r(wqk_sb[d][:, i * 128:(i + 1) * 128]),
                            _r(xt[d][:, nf * 512:(nf + 1) * 512]),
                            start=(d == 0),
                            stop=(d == DT - 1),
                        )
                for j in range(2):
                    nc.vector.tensor_copy(
                        qkT[i][:, j * 1024:(j + 1) * 1024], ps_qk[j][:]
                    )

            # v in natural layout (+ ones column for ksum)
            for ch in range(NCH):
                ps_v = pp_proj.tile([128, HL * C], F32, tag="proj", name="ps_v")
                for d in range(DT):
                    nc.tensor.matmul(
                        ps_v[:],
                        _r(xt[d][:, ch * 128:(ch + 1) * 128]),
                        _r(wv_sb[d][:]),
                        start=(d == 0),
                        stop=(d == DT - 1),
                    )
                nc.vector.tensor_copy(
                    vz[ch][:, :, 0:C],
                    ps_v.rearrange("p (h c) -> p h c", h=HL),
                )
                nc.vector.tensor_copy(
                    vz[ch][:, :, C:C + 1],
                    cst_f[:, None, 0:1].broadcast_to([128, HL, 1]),
                )

        # ---------- phase 2: per-head Performer attention ----------
        with tc.tile_pool(name="ph2", bufs=1) as ph2, \
             tc.tile_pool(name="sp2", bufs=2) as sp2, \
             tc.tile_pool(name="atp", bufs=2) as atp, \
             tc.tile_pool(name="dp1", bufs=1) as dp1:
            attnT_cur = None
            for h in [1, 0, 3, 2, 5, 4]:
                pair, half = h // 2, h % 2
                if half == 1:
                    attnT_cur = atp.tile(
                        [128, N], R32, tag="attnT", name="attnT"
                    )
                b0 = 64 * half
                qT_h = qkT[pair][b0:b0 + 64, :]
                kT_h = qkT[PAIRS + pair][b0:b0 + 64, :]
                ws_h = ws2[b0:b0 + 64, h, :]

                # stacked phi-k operand: rows 0-63 = kT_h, rows 64-127 = kT_h^2
                # (pairs with wsk rows 64-127 = -SCALE^2/2 to fuse the -|xs|^2/2
                # bias into the projection matmul)
                sq_tmp = ph2.tile([128, N], R32, tag="sqt", name="sq_tmp")
                nc.vector.tensor_mul(sq_tmp[b0:b0 + 64, :], kT_h, kT_h)
                stack = sp2.tile([128, N], R32, tag="stack", name="stack")
                nc.sync.dma_start(out=stack[0:64, :], in_=kT_h)
                nc.sync.dma_start(
                    out=stack[64:128, :], in_=sq_tmp[b0:b0 + 64, :]
                )
                wsk = sp2.tile([128, M], R32, tag="wsk", name="wsk")
                nc.vector.tensor_copy(wsk[0:64, :], ws2[0:C, h, :])
                nc.vector.tensor_copy(wsk[64:128, :], cstk[64:128, :])

                # phi_q transposed: qfT[:, mt, :] = exp(ws_h[:, mt].T @ qT_h)
                qfT = ph2.tile([128, MT, N], R32, tag="qfT", name="qfT")
                for mt in range(MT):
                    for nfh in range(2):
                        ps_q = pp_proj.tile(
                            [128, 1024], F32, tag="proj", name="ps_q"
                        )
                        for j in range(2):
                            nf = 2 * nfh + j
                            nc.tensor.matmul(
                                ps_q[:, j * 512:(j + 1) * 512],
                                _r(ws_h[:, mt * 128:(mt + 1) * 128]),
                                _r(qT_h[:, nf * 512:(nf + 1) * 512]),
                                start=True, stop=True,
                            )
                        nc.scalar.activation(
                            qfT[:, mt, nfh * 1024:(nfh + 1) * 1024],
                            ps_q[:],
                            mybir.ActivationFunctionType.Exp,
                        )

                # phi_k natural (with fused -|xs|^2/2):
                # kf[:, ch, :] = exp(stack[:, ch].T @ wsk)
                kf = ph2.tile([128, NCH, M], R32, tag="kf", name="kf")
                for ch2 in range(NCH // 2):
                    ps_k = pp_proj.tile(
                        [128, 2, 512], F32, tag="proj", name="ps_k"
                    )
                    for j in range(2):
                        ch = 2 * ch2 + j
                        nc.tensor.matmul(
                            ps_k[:, j, 0:M],
                            _r(stack[:, ch * 128:(ch + 1) * 128]),
                            _r(wsk[:]),
                            start=True, stop=True,
                        )
                    nc.scalar.activation(
                        kf[:, 2 * ch2:2 * ch2 + 2, :],
                        ps_k[:, :, 0:M],
                        mybir.ActivationFunctionType.Exp,
                    )

                # ktvT = sum_ch vz[ch].T @ kf[ch]   (C+1, M)
                pktv = pp_misc.tile([C + 1, M], F32, tag="misc", name="pktv")
                for ch in range(NCH):
                    nc.tensor.matmul(
                        pktv[:],
                        _r(vz[ch][:, h, :]),
                        _r(kf[:, ch, :]),
                        start=(ch == 0),
                        stop=(ch == NCH - 1),
                    )
                ktvTs = ph2.tile([C + 1, M], F32, tag="ktvTs", name="ktvTs")
                nc.vector.tensor_copy(ktvTs[:], pktv[:])

                # transpose to natural ktv (M, C+1) as MT (128, C+1) tiles
                ktv = ph2.tile([128, MT, C + 1], R32, tag="ktv_n", name="ktv")
                for mt in range(MT):
                    pt = pp_misc.tile([128, C + 1], F32, tag="misc", name="pt_tr")
                    nc.tensor.transpose(
                        pt[:],
                        ktvTs[:, mt * 128:(mt + 1) * 128],
                        ident[:],
                    )
                    nc.vector.tensor_copy(ktv[:, mt, :], pt[:])

                # numerator+D, divide, pack into attnT
                attn_dst = (
                    attnT_cur[0:64, :]
                    if half == 0
                    else ph2.tile([64, N], R32, tag="attn_tmp", name="attn_tmp")
                )
                for nfh in range(2):
                    nt = pp_proj.tile([C + 1, 1024], F32, tag="proj", name="nt")
                    for mt in range(MT):
                        for j in range(2):
                            nf = 2 * nfh + j
                            nc.tensor.matmul(
                                nt[:, j * 512:(j + 1) * 512],
                                _r(ktv[:, mt, :]),
                                _r(qfT[:, mt, nf * 512:(nf + 1) * 512]),
                                start=(mt == 0),
                                stop=(mt == MT - 1),
                            )
                    rec = dp1.tile([1, 1024], F32, tag="rec", name="rec")
                    nc.vector.tensor_copy(rec[0:1, :], nt[64:65, :])
                    nc.vector.reciprocal_approx_fast(rec[0:1, :], rec[0:1, :])
                    dbs = dp1.tile([64, 1024], F32, tag="dbs", name="dbs")
                    nc.gpsimd.partition_broadcast(dbs[:], rec[0:1, :])
                    nc.vector.tensor_mul(
                        attn_dst[:, nfh * 1024:(nfh + 1) * 1024],
                        nt[0:64, :],
                        dbs[:],
                    )
                if half == 1:
                    nc.sync.dma_start(
                        out=attnT_cur[64:128, :], in_=attn_dst[:]
                    )
                else:
                    # pair complete: project this pair's heads into OUT{pair}
                    for ch in range(NCH):
                        ot = outp.tile([128, DIM], F32, tag="ot", name="ot")
                        for hf in range(2):
                            po = pp_misc.tile(
                                [128, 384], F32, tag="misc", name="po"
                            )
                            nc.tensor.matmul(
                                po[:],
                                attnT_cur[:, ch * 128:(ch + 1) * 128],
                                _r(wp_sb[:, pair, hf * 384:(hf + 1) * 384]),
                                start=True, stop=True,
                            )
                            nc.vector.tensor_copy(
                                ot[:, hf * 384:(hf + 1) * 384], po[:]
                            )
                        nc.sync.dma_start(
                            out=OUTS[pair][ch * 128:(ch + 1) * 128, :],
                            in_=ot[:],
                        )


def _host_prep(x, w, Wqkv, Wproj):
    in_maps = []
    for core in range(8):
        b, g = core // 2, core % 2
        xTb = np.ascontiguousarray(x[b].T)
        Wq = Wqkv[:, g * 384:(g + 1) * 384]
        Wk = Wqkv[:, DIM + g * 384:DIM + (g + 1) * 384]
        in_maps.append(
            {
                "xT": xTb.astype(np.float32),
                "Wqk": np.ascontiguousarray(
                    np.concatenate([Wq, Wk], axis=1), dtype=np.float32
                ),
                "Wv": np.ascontiguousarray(
                    Wqkv[:, 2 * DIM + g * 384:2 * DIM + (g + 1) * 384],
                    dtype=np.float32,
                ),
                "Wm": np.ascontiguousarray(
                    w[g * HL:(g + 1) * HL] * SCALE, dtype=np.float32
                ),
                "Wp": np.ascontiguousarray(
                    Wproj[g * 384:(g + 1) * 384, :], dtype=np.float32
                ),
            }
        )
    return in_maps


def kernel(x, w, Wqkv, Wproj, bproj, _trace=False):
    from concourse.bass_utils import run_bass_kernel_spmd

    if "nc" not in _CACHED:
        _CACHED["nc"] = build_nc()
    nc = _CACHED["nc"]

    in_maps = _host_prep(
        np.asarray(x, np.float32),
        np.asarray(w, np.float32),
        np.asarray(Wqkv, np.float32),
        np.asarray(Wproj, np.float32),
    )
    res = run_bass_kernel_spmd(nc, in_maps, list(range(8)), trace=_trace)
    _CACHED["last_result"] = res

    out = np.empty((B, N, DIM), np.float32)
    for b in range(B):
        out[b] = sum(
            res.results[2 * b + g][f"OUT{p}"]
            for g in range(2) for p in range(PAIRS)
        )
    out += np.asarray(bproj, np.float32)[None, None, :]
    return out


# revision 26
# speedup vs baseline: 1.2532x; 1.2297x over previous
"""Performer (FAVOR+ linear attention) Trainium2 kernel, 8-core SPMD.

Sharding: core c -> (batch b = c//2, head-group g = c%2 of 6 heads).
Each core computes its 6 heads end-to-end plus a partial output
projection; the host sums the two head-group partials per batch and
adds bproj.

Math simplifications vs the reference (exact up to ~1e-11 rel):
  - qf = exp(proj_q): the exp(-|xs_q|^2/2) and 1/sqrt(M) factors on the
    q side cancel between numerator and denominator D.
  - kf's 1/sqrt(M) also cancels; exp(-|xs_k|^2/2) is folded into v
    (and into the ones-column that produces ksum).
  - EPS=1e-8 on D is dropped (D is >> EPS here).
"""

from contextlib import ExitStack

import ml_dtypes
import numpy as np

import concourse.bacc as bacc
import concourse.tile as tile
from concourse import mybir
from concourse.masks import make_identity

B, N, DIM = 4, 2048, 768
H, C, M = 12, 64, 384
SCALE = float(C ** (-0.25))
HL = H // 2          # heads per core
PAIRS = HL // 2      # packed head-pairs per core
NCH = N // 128       # 16 n-chunks
DT = DIM // 128      # 6 dim-tiles
MT = M // 128        # 3 feature-tiles
NF = N // 512        # 4 free-dim chunks

F32 = mybir.dt.float32
R32 = mybir.dt.float32r
BF16 = mybir.dt.bfloat16

_CACHED = {}


def _r(ap):
    """fp32r view for fp32 matmul operands; bf16/fp32r pass through."""
    return ap.bitcast(R32) if ap.dtype == F32 else ap


def build_nc():
    nc = bacc.Bacc("TRN2", target_bir_lowering=False, debug=False)

    xT = nc.dram_tensor("xT", [DIM, N], R32, kind="ExternalInput")
    Wqk = nc.dram_tensor("Wqk", [DIM, 2 * HL * C], R32, kind="ExternalInput")
    Wv = nc.dram_tensor("Wv", [DIM, HL * C], R32, kind="ExternalInput")
    Wm = nc.dram_tensor("Wm", [HL, C, M], R32, kind="ExternalInput")
    Wp = nc.dram_tensor("Wp", [HL * C, DIM], R32, kind="ExternalInput")
    OUT = nc.dram_tensor("OUT", [N, DIM], F32, kind="ExternalOutput")

    with tile.TileContext(nc) as tc:
        _build_program(tc, nc, xT, Wqk, Wv, Wm, Wp, OUT)

    nc.compile()
    return nc


def _build_program(tc, nc, xT, Wqk, Wv, Wm, Wp, OUT):
    ctx = ExitStack()
    with ctx:
        # ---------- persistent pools ----------
        singles = ctx.enter_context(tc.tile_pool(name="singles", bufs=1))
        outp = ctx.enter_context(tc.tile_pool(name="outp", bufs=2))

        ident = singles.tile([C + 1, C + 1], F32, tag="ident", name="ident")
        make_identity(nc, ident)
        cst_f = singles.tile([128, 1], F32, tag="cst_f", name="cst_f")
        nc.vector.memset(cst_f[:], 1.0)
        # rows 64-127 pair with the kT^2 half of the stacked phi-k operand
        cstk = singles.tile([128, M], F32, tag="cstk", name="cstk")
        nc.vector.memset(cstk[:], -0.5 * SCALE * SCALE)

        ws2 = singles.tile([128, HL, M], R32, tag="ws2", name="ws2")

        qkT = [
            singles.tile([128, N], R32, tag=f"qkT{i}", name=f"qkT{i}")
            for i in range(2 * PAIRS)
        ]  # 0..2 = q pairs, 3..5 = k pairs
        vz = [
            singles.tile([128, HL, C + 1], R32, tag=f"vz{ch}", name=f"vz{ch}")
            for ch in range(NCH)
        ]

        pp_proj = ctx.enter_context(
            tc.tile_pool(name="pp_proj", bufs=3, space="PSUM")
        )
        pp_misc = ctx.enter_context(
            tc.tile_pool(name="pp_misc", bufs=2, space="PSUM")
        )

        # ---------- phase 1: qkv projection ----------
        with tc.tile_pool(name="ph1", bufs=1) as ph1:
            xt = [
                ph1.tile([128, N], R32, tag=f"xt{d}", name=f"xt{d}")
                for d in range(DT)
            ]
            wqk_sb = [
                ph1.tile([128, 2 * HL * C], R32, tag=f"wqk{d}", name=f"wqk{d}")
                for d in range(DT)
            ]
            wv_sb = [
                ph1.tile([128, HL * C], R32, tag=f"wv{d}", name=f"wv{d}")
                for d in range(DT)
            ]
            for d in range(DT):
                nc.sync.dma_start(out=xt[d][:], in_=xT[d * 128:(d + 1) * 128, :])
                nc.sync.dma_start(out=wqk_sb[d][:], in_=Wqk[d * 128:(d + 1) * 128, :])
            for d in range(DT):
                nc.sync.dma_start(out=wv_sb[d][:], in_=Wv[d * 128:(d + 1) * 128, :])
            for h in range(HL):
                nc.sync.dma_start(out=ws2[0:C, h, :], in_=Wm[h])
                nc.sync.dma_start(out=ws2[C:128, h, :], in_=Wm[h])

            # q/k head-pairs: qkT[i] = Wqk[:, 128i:128i+128].T @ x.T  (128, N)
            for i in range(2 * PAIRS):
                ps_qk = [
                    pp_proj.tile([128, 1024], F32, tag="proj", name=f"ps_qk{j}")
                    for j in range(2)
                ]
                for d in range(DT):
                    for nf in range(NF):
                        nc.tensor.matmul(
                            ps_qk[nf // 2][:, (nf % 2) * 512:(nf % 2 + 1) * 512],
                            _r(wqk_sb[d][:, i * 128:(i + 1) * 128]),
                            _r(xt[d][:, nf * 512:(nf + 1) * 512]),
                            start=(d == 0),
                            stop=(d == DT - 1),
                        )
                for j in range(2):
                    nc.vector.tensor_copy(
                        qkT[i][:, j * 1024:(j + 1) * 1024], ps_qk[j][:]
                    )

            # v in natural layout (+ ones column for ksum)
            for ch in range(NCH):
                ps_v = pp_proj.tile([128, HL * C], F32, tag="proj", name="ps_v")
                for d in range(DT):
                    nc.tensor.matmul(
                        ps_v[:],
                        _r(xt[d][:, ch * 128:(ch + 1) * 128]),
                        _r(wv_sb[d][:]),
                        start=(d == 0),
                        stop=(d == DT - 1),
                    )
                nc.vector.tensor_copy(
                    vz[ch][:, :, 0:C],
                    ps_v.rearrange("p (h c) -> p h c", h=HL),
                )
                nc.vector.tensor_copy(
                    vz[ch][:, :, C:C + 1],
                    cst_f[:, None, 0:1].broadcast_to([128, HL, 1]),
                )

        # ---------- phase 2: per-head Performer attention ----------
        with tc.tile_pool(name="ph2", bufs=1) as ph2, \
             tc.tile_pool(name="sp2", bufs=2) as sp2, \
             tc.tile_pool(name="atp", bufs=3) as atp, \
             tc.tile_pool(name="dp1", bufs=1) as dp1:
            attnT_cur = None
            attnT_all = []
            for h in [1, 0, 3, 2, 5, 4]:
                pair, half = h // 2, h % 2
                if half == 1:
                    attnT_cur = atp.tile(
                        [128, N], R32, tag="attnT", name="attnT"
                    )
                    attnT_all.append(attnT_cur)
                b0 = 64 * half
                qT_h = qkT[pair][b0:b0 + 64, :]
                kT_h = qkT[PAIRS + pair][b0:b0 + 64, :]
                ws_h = ws2[b0:b0 + 64, h, :]

                # stacked phi-k operand: rows 0-63 = kT_h, rows 64-127 = kT_h^2
                # (pairs with wsk rows 64-127 = -SCALE^2/2 to fuse the -|xs|^2/2
                # bias into the projection matmul)
                sq_tmp = ph2.tile([128, N], R32, tag="sqt", name="sq_tmp")
                nc.vector.tensor_mul(sq_tmp[b0:b0 + 64, :], kT_h, kT_h)
                stack = sp2.tile([128, N], R32, tag="stack", name="stack")
                nc.sync.dma_start(out=stack[0:64, :], in_=kT_h)
                nc.sync.dma_start(
                    out=stack[64:128, :], in_=sq_tmp[b0:b0 + 64, :]
                )
                wsk = sp2.tile([128, M], R32, tag="wsk", name="wsk")
                nc.vector.tensor_copy(wsk[0:64, :], ws2[0:C, h, :])
                nc.vector.tensor_copy(wsk[64:128, :], cstk[64:128, :])

                # phi_q transposed: qfT[:, mt, :] = exp(ws_h[:, mt].T @ qT_h)
                qfT = ph2.tile([128, MT, N], R32, tag="qfT", name="qfT")
                for mt in range(MT):
                    for nfh in range(2):
                        ps_q = pp_proj.tile(
                            [128, 1024], F32, tag="proj", name="ps_q"
                        )
                        for j in range(2):
                            nf = 2 * nfh + j
                            nc.tensor.matmul(
                                ps_q[:, j * 512:(j + 1) * 512],
                                _r(ws_h[:, mt * 128:(mt + 1) * 128]),
                                _r(qT_h[:, nf * 512:(nf + 1) * 512]),
                                start=True, stop=True,
                            )
                        nc.scalar.activation(
                            qfT[:, mt, nfh * 1024:(nfh + 1) * 1024],
                            ps_q[:],
                            mybir.ActivationFunctionType.Exp,
                        )

                # phi_k natural (with fused -|xs|^2/2):
                # kf[:, ch, :] = exp(stack[:, ch].T @ wsk)
                kf = ph2.tile([128, NCH, M], R32, tag="kf", name="kf")
                for ch2 in range(NCH // 2):
                    ps_k = pp_proj.tile(
                        [128, 2, 512], F32, tag="proj", name="ps_k"
                    )
                    for j in range(2):
                        ch = 2 * ch2 + j
                        nc.tensor.matmul(
                            ps_k[:, j, 0:M],
                            _r(stack[:, ch * 128:(ch + 1) * 128]),
                            _r(wsk[:]),
                            start=True, stop=True,
                        )
                    nc.scalar.activation(
                        kf[:, 2 * ch2:2 * ch2 + 2, :],
                        ps_k[:, :, 0:M],
                        mybir.ActivationFunctionType.Exp,
                    )

                # ktvT = sum_ch vz[ch].T @ kf[ch]   (C+1, M)
                pktv = pp_misc.tile([C + 1, M], F32, tag="misc", name="pktv")
                for ch in range(NCH):
                    nc.tensor.matmul(
                        pktv[:],
                        _r(vz[ch][:, h, :]),
                        _r(kf[:, ch, :]),
                        start=(ch == 0),
                        stop=(ch == NCH - 1),
                    )
                ktvTs = ph2.tile([C + 1, M], F32, tag="ktvTs", name="ktvTs")
                nc.vector.tensor_copy(ktvTs[:], pktv[:])

                # transpose to natural ktv (M, C+1) as MT (128, C+1) tiles
                ktv = ph2.tile([128, MT, C + 1], R32, tag="ktv_n", name="ktv")
                for mt in range(MT):
                    pt = pp_misc.tile([128, C + 1], F32, tag="misc", name="pt_tr")
                    nc.tensor.transpose(
                        pt[:],
                        ktvTs[:, mt * 128:(mt + 1) * 128],
                        ident[:],
                    )
                    nc.vector.tensor_copy(ktv[:, mt, :], pt[:])

                # numerator+D, divide, pack into attnT
                attn_dst = (
                    attnT_cur[0:64, :]
                    if half == 0
                    else ph2.tile([64, N], R32, tag="attn_tmp", name="attn_tmp")
                )
                for nfh in range(2):
                    nt = pp_proj.tile([C + 1, 1024], F32, tag="proj", name="nt")
                    for mt in range(MT):
                        for j in range(2):
                            nf = 2 * nfh + j
                            nc.tensor.matmul(
                                nt[:, j * 512:(j + 1) * 512],
                                _r(ktv[:, mt, :]),
                                _r(qfT[:, mt, nf * 512:(nf + 1) * 512]),
                                start=(mt == 0),
                                stop=(mt == MT - 1),
                            )
                    rec = dp1.tile([1, 1024], F32, tag="rec", name="rec")
                    nc.vector.tensor_copy(rec[0:1, :], nt[64:65, :])
                    nc.vector.reciprocal_approx_fast(rec[0:1, :], rec[0:1, :])
                    dbs = dp1.tile([64, 1024], F32, tag="dbs", name="dbs")
                    nc.gpsimd.partition_broadcast(dbs[:], rec[0:1, :])
                    nc.vector.tensor_mul(
                        attn_dst[:, nfh * 1024:(nfh + 1) * 1024],
                        nt[0:64, :],
                        dbs[:],
                    )
                if half == 1:
                    nc.sync.dma_start(
                        out=attnT_cur[64:128, :], in_=attn_dst[:]
                    )

        # ---------- phase 3: output projection ----------
        with tc.tile_pool(name="ph3", bufs=1) as ph3:
            wp_sb = ph3.tile([128, PAIRS, DIM], R32, tag="wp", name="wp_sb")
            for p in range(PAIRS):
                nc.sync.dma_start(
                    out=wp_sb[:, p, :], in_=Wp[p * 128:(p + 1) * 128, :]
                )
            for ch in range(NCH):
                ot = outp.tile([128, DIM], F32, tag="ot", name="ot")
                for hf in range(2):
                    po = pp_misc.tile([128, 384], F32, tag="misc", name="po")
                    for p in range(PAIRS):
                        nc.tensor.matmul(
                            po[:],
                            attnT_all[p][:, ch * 128:(ch + 1) * 128],
                            _r(wp_sb[:, p, hf * 384:(hf + 1) * 384]),
                            start=(p == 0),
                            stop=(p == PAIRS - 1),
                        )
                    nc.vector.tensor_copy(ot[:, hf * 384:(hf + 1) * 384], po[:])
                nc.sync.dma_start(
                    out=OUT[ch * 128:(ch + 1) * 128, :], in_=ot[:]
                )

def _host_prep(x, w, Wqkv, Wproj):
    in_maps = []
    for core in range(8):
        b, g = core // 2, core % 2
        xTb = np.ascontiguousarray(x[b].T)
        Wq = Wqkv[:, g * 384:(g + 1) * 384]
        Wk = Wqkv[:, DIM + g * 384:DIM + (g + 1) * 384]
        in_maps.append(
            {
                "xT": xTb.astype(np.float32),
                "Wqk": np.ascontiguousarray(
                    np.concatenate([Wq, Wk], axis=1), dtype=np.float32
                ),
                "Wv": np.ascontiguousarray(
                    Wqkv[:, 2 * DIM + g * 384:2 * DIM + (g + 1) * 384],
                    dtype=np.float32,
                ),
                "Wm": np.ascontiguousarray(
                    w[g * HL:(g + 1) * HL] * SCALE, dtype=np.float32
                ),
                "Wp": np.ascontiguousarray(
                    Wproj[g * 384:(g + 1) * 384, :], dtype=np.float32
                ),
            }
        )
    return in_maps


def kernel(x, w, Wqkv, Wproj, bproj, _trace=False):
    from concourse.bass_utils import run_bass_kernel_spmd

    if "nc" not in _CACHED:
        _CACHED["nc"] = build_nc()
    nc = _CACHED["nc"]

    in_maps = _host_prep(
        np.asarray(x, np.float32),
        np.asarray(w, np.float32),
        np.asarray(Wqkv, np.float32),
        np.asarray(Wproj, np.float32),
    )
    res = run_bass_kernel_spmd(nc, in_maps, list(range(8)), trace=_trace)
    _CACHED["last_result"] = res

    out = np.empty((B, N, DIM), np.float32)
    for b in range(B):
        out[b] = res.results[2 * b]["OUT"] + res.results[2 * b + 1]["OUT"]
    out += np.asarray(bproj, np.float32)[None, None, :]
    return out
